# revision 1
# baseline (speedup 1.0000x reference)
"""IsoMaxPlus distance head on 8 NeuronCores.

out[n, c] = -|ds| * sqrt(max(2 - 2 * <f_n/|f_n|, p_c/|p_c|>, eps))

Data-parallel over the batch axis: features rows sharded 8 ways, prototypes and
distance_scale replicated (matches the sharding hint). The core matmul runs in
bf16 on the tensor engine (full rate; fp32 matmul is 4x slower), with fp32
norms/epilogue so the only precision loss is the bf16 rounding of the
normalized operands (~5e-5 relative on the output).
"""

import functools

import numpy as np

import jax
import jax.numpy as jnp
from jax.sharding import Mesh, NamedSharding, PartitionSpec as P

N_CORES = 8
EPS_NORM = 1e-12
EPS_SQ = 1e-12


def _normalize(x):
    n = jnp.sqrt(jnp.sum(x * x, axis=-1, keepdims=True))
    return x / jnp.maximum(n, EPS_NORM)


def _shard_fn(f, p, ds):
    # f: [N/8, D] local shard; p: [C, D] replicated; ds: [1] replicated
    fn = _normalize(f).astype(jnp.bfloat16)
    pn = _normalize(p).astype(jnp.bfloat16)
    sim = jax.lax.dot_general(
        fn, pn,
        dimension_numbers=(((1,), (1,)), ((), ())),
        preferred_element_type=jnp.float32,
    )
    sq = jnp.maximum(2.0 - 2.0 * sim, EPS_SQ)
    return -jnp.abs(ds[0]) * jnp.sqrt(sq)


@functools.cache
def _jitted():
    devices = jax.devices()[:N_CORES]
    mesh = Mesh(np.asarray(devices), ("core",))
    fn = jax.jit(
        jax.shard_map(
            _shard_fn,
            mesh=mesh,
            in_specs=(P("core"), P(), P()),
            out_specs=P("core"),
        ),
        in_shardings=(
            NamedSharding(mesh, P("core")),
            NamedSharding(mesh, P()),
            NamedSharding(mesh, P()),
        ),
    )
    return fn


def kernel(features, prototypes, distance_scale):
    features = np.ascontiguousarray(features, dtype=np.float32)
    prototypes = np.ascontiguousarray(prototypes, dtype=np.float32)
    distance_scale = np.ascontiguousarray(distance_scale, dtype=np.float32)
    out = _jitted()(features, prototypes, distance_scale)
    return np.asarray(jax.device_get(out)).astype(np.float32)

